# revision 9
# baseline (speedup 1.0000x reference)
"""Trainium2 Bass kernel for llama-style GQA attention layer (B=2, S=1024,
D=4096, H=32, KVH=8, HD=128, start_pos=0), tensor-parallel over heads on 8
NeuronCores.

Per-core plan (core c of 8):
  - owns q heads 4c..4c+3 (wq cols 512c..512c+512) and kv head c
    (wk/wv cols 128c..128c+128); GQA grouping means exactly its heads use
    exactly its kv head.
  - computes qT/kT/vT = (x @ w).T directly in [col, token] layout via
    w-stationary matmuls against host-transposed xT.
  - RoPE applied in qT/kT layout; the head-dim interleave is de-interleaved
    by permuting wq/wk columns on the host.  The half-swap runs as an
    SBUF->SBUF partition-crossing DMA; the RoPE result is written to bf16
    tiles (the whole score path q/k/scores runs in bf16 - the q.k
    contraction over 128 averages the quantization noise to ~3e-3 rel).
  - attention in transposed orientation: ST[k, q] from kT-stationary
    matmuls, softmax without max-subtraction, PV with v-stationary matmuls
    giving outT [hd, q].  Per head: all score matmuls + exps first, then
    the softmax denominators as ONE ones-stationary accumulation chain
    (single LDWEIGHTS, and the result broadcast across partitions for
    free), then the reciprocal on the DVE overlapped with the PV chain.
  - per 1024-token pair of chunks, heads' outT concat -> AllGather (2
    collectives total instead of 4: halves the ncfw floors, keeps a long
    compute window) -> full attn_outT on every core -> each core computes
    a 512-column slice of the wo projection in bf16.
Host unshards by concatenating the 8 column slices and transposing.

Emitted as ONE interleaved stream: for each 512-token chunk, projection ->
RoPE -> attention -> (per pair) AllGather, with wo at the end; the Tile
scheduler fills dependency stalls across phase boundaries.  PSUM is split
4/2/2 banks: projection's 6 output columns run in two groups of 3 sharing
a 4-slot accumulator tag with the wo units; chunk 0 borrows the idle score
banks to run all 6 columns in one pass.  The xs (xT) stream for a chunk is
loaded into SBUF once and the second column-group re-reads it from SBUF
(saves 12 MB of HBM traffic).  Weight loads ride the Activation HWDGE
queue so the startup xs stream has the SP queue to itself.

Projection and wo matmuls run in bf16 (fp32 PSUM accumulation); scores are
bf16 with fp32 exp; y is written bf16 and upcast on the host.
"""

import numpy as np
import concourse.mybir as mybir
import concourse.tile as tile
from concourse import bacc
from concourse.bass_utils import run_bass_kernel_spmd

N_CORES = 8
B, S, D = 2, 1024, 4096
TOK = B * S            # 2048 flattened tokens
HD = 128
NHC = 4                # q heads per core
CH = 4                 # 512-token chunks
F32 = mybir.dt.float32
F32R = mybir.dt.float32r
BF16 = mybir.dt.bfloat16
USE_BF16 = True          # bf16 projection/wo matmuls (fp32 PSUM accumulate)
WDT = BF16 if USE_BF16 else F32R
SCALE = 1.0 / float(np.sqrt(128.0))

_CACHE = {}


def _emit(nc, tc, aps, collective=True, phases=3, sfx=""):
    xt, wqkv, wo, cost, sint, mask, iden, ones, y = aps
    # one AllGather per 1024-token chunk pair (= per batch): fires as soon
    # as that pair's attention completes, with the following chunks'
    # projection/attention + earlier wo units as its compute window.
    ag_in = [nc.dram_tensor(f"ag_in{p}{sfx}", [1024, 512], WDT) for p in range(2)]
    ag_out = [
        nc.dram_tensor(f"ag_out{p}{sfx}", [8 * 1024, 512], WDT, addr_space="Shared")
        for p in range(2)
    ]
    EXP = mybir.ActivationFunctionType.Exp

    with tc.tile_pool(name="pp" + sfx, bufs=1) as pp, tc.tile_pool(
        name="ps" + sfx, bufs=1, space="PSUM"
    ) as ps:
        mask_sb = pp.tile([128, 128], BF16, tag="mask")
        iden_sb = pp.tile([128, 128], BF16, tag="iden")
        ones_sb = pp.tile([128, 128], BF16, tag="ones")
        nc.gpsimd.dma_start(out=mask_sb, in_=mask[:, :])
        nc.gpsimd.dma_start(out=iden_sb, in_=iden[:, :])
        nc.gpsimd.dma_start(out=ones_sb, in_=ones[:, :])

        # persistent SBUF state: RoPE'd q (4 heads) + k, bf16, per chunk
        qkv = [
            [pp.tile([128, 512], BF16, tag=f"qkv{c}_{t}", name=f"qkv{c}_{t}") for t in range(CH)]
            for c in range(5)
        ]
        vsb = {}  # (b, j) -> [tok128, hd128] bf16 v blocks
        wt = [None] * 8
        wt2 = [pp.tile([128, 4, 512], WDT, tag=f"wo{jg}", name=f"wo{jg}") for jg in range(8)]

        def proj_chunk(t):
            tsl = slice(t * 512, (t + 1) * 512)
            cos_sb = pp.tile([128, 512], F32R, tag="cos", bufs=2)
            sin_sb = pp.tile([128, 512], F32R, tag="sin", bufs=2)
            # Pool/SWDGE queue: keeps the SP (xs) and ACT (weights) HWDGE
            # queues clear at chunk starts
            nc.gpsimd.dma_start(out=cos_sb, in_=cost[:, tsl].bitcast(F32R))
            nc.gpsimd.dma_start(out=sin_sb, in_=sint[:, tsl].bitcast(F32R))
            vtmp = pp.tile([128, 512], BF16, tag="vtmp", bufs=2, name=f"vtmp{t}")
            # Chunk 0 runs all 6 output columns in one pass over xT
            # (borrowing 2 idle score banks — attention hasn't started yet)
            # to halve the DMA-bound startup window.  Later chunks use two
            # col-groups of 3 so only 4 accumulator banks are needed while
            # attention/wo own the rest; the xs tiles are kept in SBUF so
            # the second group re-reads them without touching HBM.
            groups = [(0, 1, 2, 3, 4, 5)] if t == 0 else [(0, 1, 2), (3, 4, 5)]
            xs_t = [None] * 16
            for gi, cols in enumerate(groups):
                psl = {
                    c: ps.tile(
                        [128, 512], F32,
                        tag=("acc" if c < 4 or t > 0 else "st"),
                        bufs=(4 if c < 4 or t > 0 else 2),
                        name=f"psl{t}_{c}",
                    )
                    for c in cols
                }
                for jg2 in range(16):
                    def load_w_part(jg, part, rows, eng):
                        w_ = pp.tile(
                            [128, rows // 128, 768], WDT,
                            tag=f"w{jg}_{part}", name=f"w{jg}_{part}",
                        )
                        eng.dma_start(
                            out=w_,
                            in_=wqkv[
                                jg * 512 + part * rows : jg * 512
                                + part * rows + rows,
                                :,
                            ]
                            .rearrange("(jj p) n -> p jj n", p=128)
                            .bitcast(WDT),
                        )
                        for sub in range(rows // 128):
                            wt[jg].append((w_, sub))

                    if gi == 0:
                        xs = pp.tile([128, 2, 512], WDT, tag="xs", bufs=16,
                                     name=f"xs{t}_{jg2}")
                        src = (
                            xt[jg2 * 256 : (jg2 + 1) * 256, tsl]
                            .rearrange("(jj p) s -> p jj s", p=128)
                            .bitcast(WDT)
                        )
                        if t == 0 and jg2 == 0:
                            # startup interleave on the SP queue: the first
                            # matmul waits only on w0 part 0 + the first xs
                            # half; everything else streams in behind
                            wt[0] = []
                            load_w_part(0, 0, 128, nc.sync)
                            nc.sync.dma_start(out=xs[:, 0, :], in_=src[:, 0, :])
                            load_w_part(0, 1, 128, nc.sync)
                            nc.sync.dma_start(out=xs[:, 1, :], in_=src[:, 1, :])
                            load_w_part(0, 2, 128, nc.scalar)
                            load_w_part(0, 3, 128, nc.scalar)
                        else:
                            nc.sync.dma_start(out=xs, in_=src)
                        xs_t[jg2] = xs
                    else:
                        xs = xs_t[jg2]
                    if t == 0 and jg2 % 2 == 0 and jg2 > 0:
                        # wqkv stationary tiles as independent pieces so a
                        # matmul only waits on the slice it reads
                        jg = jg2 // 2
                        wt[jg] = []
                        for part in range(2):
                            load_w_part(jg, part, 256, nc.scalar)
                    for jj in range(2):
                        jga, jja = jg2 // 2, (jg2 % 2) * 2 + jj
                        w_, sub = wt[jga][jja]
                        for c in cols:
                            nc.tensor.matmul(
                                psl[c],
                                w_[:, sub, c * 128 : (c + 1) * 128],
                                xs[:, jj, :],
                                start=(jg2 == 0 and jj == 0),
                                stop=(jg2 == 15 and jj == 1),
                            )
                for c in cols:
                    if c < 5:
                        # RoPE on the 4 q heads + k (v stays raw).  Rows
                        # 0:64 hold the de-interleaved even (real) lanes,
                        # 64:128 the odd (imag) lanes.  out = x*[c;c] +
                        # swap(x)*[-s;s]; the half-swap crosses partitions
                        # so it runs as an SBUF->SBUF DMA.  The final add
                        # writes the persistent bf16 tile.
                        qraw = pp.tile([128, 512], F32R, tag="qraw", bufs=3,
                                       name=f"qraw{t}_{c}")
                        nc.vector.tensor_copy(qraw, psl[c].bitcast(F32R))
                        xsw = pp.tile([128, 512], F32R, tag="xsw", bufs=2)
                        nc.sync.dma_start(out=xsw[0:64, :], in_=qraw[64:128, :])
                        nc.sync.dma_start(out=xsw[64:128, :], in_=qraw[0:64, :])
                        m1 = pp.tile([128, 512], F32R, tag="m1", bufs=2)
                        m2 = pp.tile([128, 512], F32R, tag="m2", bufs=2)
                        nc.vector.tensor_mul(m1, qraw, cos_sb)
                        nc.vector.tensor_mul(m2, xsw, sin_sb)
                        nc.vector.tensor_add(qkv[c][t], m1, m2)
                    else:
                        nc.vector.tensor_copy(vtmp, psl[c])
            # v in natural [token, hd] layout via PE transposes
            b, t2 = t // 2, t % 2
            for jj in range(4):
                trp = ps.tile([128, 128], BF16, tag="st", bufs=2)
                nc.tensor.transpose(
                    trp, vtmp[:, jj * 128 : (jj + 1) * 128], iden_sb
                )
                v_ = pp.tile([128, 128], BF16, tag="vsb", bufs=16,
                             name=f"vsb{t}_{jj}")
                nc.vector.tensor_copy(v_, trp)
                vsb[(b, t2 * 4 + jj)] = v_

        def attn_unit(t):
            b, t2 = t // 2, t % 2
            jmax = (t2 + 1) * 4
            for h in range(NHC):
                outp = ps.tile([128, 512], F32, tag="ov", bufs=2,
                               name=f"outp{t}_{h}")
                sums = ps.tile([128, 512], F32, tag="ov", bufs=2,
                               name=f"sums{t}_{h}")
                qt = qkv[h][t]
                blocks = []
                for j in range(jmax):
                    kt = qkv[4][b * 2 + j // 4]
                    qstart = max(t2 * 512, j * 128)   # in batch tokens
                    width = t2 * 512 + 512 - qstart
                    qoff = qstart - t2 * 512          # within chunk
                    st = ps.tile([128, 512], F32, tag="st", bufs=2)
                    nc.tensor.matmul(
                        st[:, :width],
                        kt[:, (j % 4) * 128 : (j % 4 + 1) * 128],
                        qt[:, qoff : qoff + width],
                        start=True,
                        stop=True,
                    )
                    pexp = pp.tile([128, 512], BF16, tag="pexp", bufs=10)
                    nc.scalar.activation(
                        pexp[:, :width], st[:, :width], EXP, scale=SCALE
                    )
                    if j * 128 >= t2 * 512:
                        # zero the non-causal lower triangle of the
                        # diagonal block (mask_sb is 0/1 here)
                        nc.vector.tensor_mul(
                            pexp[:, 0:128], pexp[:, 0:128], mask_sb
                        )
                    blocks.append((j, qoff, width, pexp))
                # softmax denominators: one ones-stationary accumulation
                # chain (single LDWEIGHTS for the whole head); the matmul
                # output IS the row-sum broadcast across partitions.
                for j, qoff, width, pexp in blocks:
                    nc.tensor.matmul(
                        sums[:, qoff : qoff + width],
                        ones_sb,
                        pexp[:, :width],
                        start=(j == 0),
                        stop=(j == jmax - 1),
                    )
                # reciprocal runs on the DVE while the PE streams the PV
                # chain below
                rec = pp.tile([128, 512], F32, tag="rec", bufs=2)
                nc.vector.reciprocal(rec, sums)
                for j, qoff, width, pexp in blocks:
                    nc.tensor.matmul(
                        outp[:, qoff : qoff + width],
                        vsb[(b, j)],
                        pexp[:, :width],
                        start=(j == 0),
                        stop=(j == jmax - 1),
                    )
                aot = pp.tile([128, 512], WDT, tag="aot", bufs=4,
                              name=f"aot{t}_{h}")
                nc.vector.tensor_mul(aot, outp, rec)
                # stage this head's AllGather input immediately so the
                # collective only waits on the last head's normalization
                nc.sync.dma_start(
                    out=ag_in[b][t2 * 512 + h * 128 : t2 * 512 + (h + 1) * 128, :],
                    in_=aot,
                )

        def ag_pair(p):
            if collective:
                nc.gpsimd.collective_compute(
                    "AllGather",
                    mybir.AluOpType.bypass,
                    ins=[ag_in[p][:, :]],
                    outs=[ag_out[p][:, :]],
                    replica_groups=[list(range(N_CORES))],
                )
            else:
                nc.gpsimd.dma_start(out=ag_out[p][0:1024, :], in_=ag_in[p][:, :])

        def wo_unit(b, t2):
            yp = [
                ps.tile([128, 512], F32, tag="acc", bufs=4, name=f"yp{b}{t2}_{d}")
                for d in range(4)
            ]
            for jg in range(8):
                ags = pp.tile([128, 4, 512], WDT, tag="ags", bufs=4)
                nc.scalar.dma_start(
                    out=ags,
                    in_=ag_out[b][
                        jg * 1024 + t2 * 512 : jg * 1024 + (t2 + 1) * 512, :
                    ].rearrange("(jj p) s -> p jj s", p=128),
                )
                for jjj in range(4):
                    for d in range(4):
                        nc.tensor.matmul(
                            yp[d],
                            wt2[jg][:, jjj, d * 128 : (d + 1) * 128],
                            ags[:, jjj, :],
                            start=(jg == 0 and jjj == 0),
                            stop=(jg == 7 and jjj == 3),
                        )
            last = b == 1 and t2 == 1
            for d in range(4):
                ys = pp.tile([128, 512], BF16, tag="ys", bufs=4)
                if d % 2 == 0:
                    nc.vector.tensor_copy(ys, yp[d])
                else:
                    nc.scalar.copy(ys, yp[d])
                eng = nc.scalar if (last and d % 2) else nc.sync
                eng.dma_start(
                    out=y[
                        d * 128 : (d + 1) * 128,
                        b * 1024 + t2 * 512 : b * 1024 + (t2 + 1) * 512,
                    ],
                    in_=ys,
                )

        proj_chunk(0)
        if phases >= 2:
            attn_unit(0)
        proj_chunk(1)
        if phases >= 2:
            attn_unit(1)
            ag_pair(0)
        if phases >= 3:
            for jg in range(8):
                nc.scalar.dma_start(
                    out=wt2[jg],
                    in_=wo[jg * 512 : (jg + 1) * 512, :]
                    .rearrange("(jj p) n -> p jj n", p=128)
                    .bitcast(WDT),
                )
        proj_chunk(2)
        if phases >= 2:
            attn_unit(2)
        proj_chunk(3)
        if phases >= 2:
            attn_unit(3)
            ag_pair(1)
        if phases >= 3:
            for b in range(B):
                for t2 in range(2):
                    wo_unit(b, t2)


def _build(single=False, phases=3):
    key = ("nc_single" if single else "nc") + str(phases)
    if key in _CACHE:
        return _CACHE[key]
    nc = bacc.Bacc(
        "TRN2",
        target_bir_lowering=False,
        debug=False,
        num_devices=1 if single else N_CORES,
    )
    xt = nc.declare_dram_parameter("xt", [D, TOK], WDT, isOutput=False)
    wqkv = nc.declare_dram_parameter("wqkv", [D, 768], WDT, isOutput=False)
    wo = nc.declare_dram_parameter("wo", [D, 512], WDT, isOutput=False)
    cost = nc.declare_dram_parameter("cost", [128, TOK], F32, isOutput=False)
    sint = nc.declare_dram_parameter("sint", [128, TOK], F32, isOutput=False)
    mask = nc.declare_dram_parameter("mask", [128, 128], BF16, isOutput=False)
    iden = nc.declare_dram_parameter("iden", [128, 128], BF16, isOutput=False)
    ones = nc.declare_dram_parameter("ones", [128, 128], BF16, isOutput=False)
    y = nc.declare_dram_parameter("y", [512, TOK], BF16, isOutput=True)
    with tile.TileContext(nc) as tc:
        _emit(
            nc,
            tc,
            (xt, wqkv, wo, cost, sint, mask, iden, ones, y),
            collective=not single,
            phases=phases,
        )
    nc.compile()
    _CACHE[key] = nc
    return nc


def _host_inputs(x, wq, wk, wv, wo, freqs_cos, freqs_sin):
    """Build the per-core input maps (host-side sharding/layout prep)."""
    import ml_dtypes

    wnp = ml_dtypes.bfloat16 if USE_BF16 else np.float32
    xt = np.ascontiguousarray(x.reshape(TOK, D).T).astype(wnp)  # [D, TOK]
    # de-interleave permutation of the head dim for q/k weight columns
    perm = np.concatenate([np.arange(0, HD, 2), np.arange(1, HD, 2)])
    cos_t = np.tile(freqs_cos.T, (1, B))  # [64, TOK]
    sin_t = np.tile(freqs_sin.T, (1, B))
    cost = np.concatenate([cos_t, cos_t], axis=0).astype(np.float32)  # [128, TOK]
    sint = np.concatenate([-sin_t, sin_t], axis=0).astype(np.float32)
    kq, qq = np.meshgrid(np.arange(128), np.arange(128), indexing="ij")
    mask = np.where(qq >= kq, 1.0, 0.0).astype(wnp)  # [k, q], 0/1
    iden = np.eye(128, dtype=wnp)
    ones = np.ones((128, 128), wnp)

    in_maps = []
    for c in range(N_CORES):
        wq_c = wq[:, c * 512 : (c + 1) * 512].reshape(D, NHC, HD)[:, :, perm]
        wq_c = wq_c.reshape(D, NHC * HD)
        wk_c = wk[:, c * 128 : (c + 1) * 128][:, perm]
        wv_c = wv[:, c * 128 : (c + 1) * 128]
        wqkv_c = np.ascontiguousarray(
            np.concatenate([wq_c, wk_c, wv_c], axis=1)
        ).astype(wnp)  # [D, 768]
        wo_c = np.ascontiguousarray(wo[:, c * 512 : (c + 1) * 512]).astype(wnp)
        in_maps.append(
            {
                "xt": xt,
                "wqkv": wqkv_c,
                "wo": wo_c,
                "cost": cost,
                "sint": sint,
                "mask": mask,
                "iden": iden,
                "ones": ones,
            }
        )
    return in_maps


def kernel(
    x,
    wq,
    wk,
    wv,
    wo,
    freqs_cos,
    freqs_sin,
    cache_k=None,
    cache_v=None,
    start_pos=0,
):
    # start_pos is 0 in this problem; the cache read-back region is then
    # exactly the freshly written k/v, so the caches never matter.
    assert int(start_pos) == 0
    x = np.asarray(x, np.float32)
    in_maps = _host_inputs(
        x,
        np.asarray(wq, np.float32),
        np.asarray(wk, np.float32),
        np.asarray(wv, np.float32),
        np.asarray(wo, np.float32),
        np.asarray(freqs_cos, np.float32),
        np.asarray(freqs_sin, np.float32),
    )
    nc = _build()
    res = run_bass_kernel_spmd(nc, in_maps, list(range(N_CORES))).results
    y_t = np.concatenate(
        [res[c]["y"].astype(np.float32) for c in range(N_CORES)], axis=0
    )  # [D, TOK]
    return np.ascontiguousarray(y_t.T).reshape(B, S, D).astype(np.float32)
